# revision 3
# baseline (speedup 1.0000x reference)
"""Trainium2 Bass kernel for nn_Conv2D_BinaryLayer (3x3 VALID conv, binarized
weights, bias add).

  x      [32, 112, 112, 128] f32  (NHWC)
  kernel [3, 3, 128, 256]    f32  -> binarized to {-1, +1} (bf16, exact)
  bias   [256]               f32
  out    [32, 110, 110, 256] f32

Strategy: data-parallel over batch, 4 images per NeuronCore on 8 cores.
Host prepares layouts (sharding/packing, no conv math): x is cast to bf16
and transposed per image to xT[ci, h*W+w]; weights are binarized (+/-1,
exact in bf16) and repacked [ci, (tap co)]; the output comes back
[co, pix] bf16 and is transposed back to NHWC f32 on the host.

On device the PE does nothing but conv matmuls at the bf16 roofline
(~217ns per 512-col matmul, LDWEIGHTS hidden under the stream):
psum[co_half, pix_block] accumulates 9 taps of
wb[ci, co_half].T @ xT[ci, pix_slice], then the bias add is fused into
the PSUM->SBUF copy (alternating DVE/ACT), stored bf16 (output tolerance
2e-2 >> bf16 rounding).
"""

import os
import sys
import types
import numpy as np
import ml_dtypes
from contextlib import ExitStack

# bass_utils' trace path (taken when the caller sets BASS_TRACE) imports
# antenv.axon_hooks, which this image's antenv lacks. Register a no-op
# fallback so that path degrades to an untraced run instead of crashing.
try:
    import antenv.axon_hooks  # noqa: F401
except ImportError:
    try:
        import antenv
        _hooks = types.ModuleType("antenv.axon_hooks")
        _hooks.get_axon_ntff_profile_hook = lambda: None
        _hooks.set_axon_ntff_profile_hook = lambda h: None
        sys.modules.setdefault("antenv.axon_hooks", _hooks)
        if not hasattr(antenv, "axon_hooks"):
            antenv.axon_hooks = sys.modules["antenv.axon_hooks"]
    except Exception:
        pass

import concourse.bass as bass
import concourse.tile as tile
from concourse import mybir
from concourse.bass_utils import run_bass_kernel_spmd

# ---------------------------------------------------------------- shapes
N, H, W, CIN, COUT = 32, 112, 112, 128, 256
KH = KW = 3
HO, WO = H - KH + 1, W - KW + 1  # 110, 110
N_CORES = 8
NPC = N // N_CORES               # images per core = 4
PIX = H * W                      # 12544
NTAP = KH * KW                   # 9

# Conv output blocks over the flat 112-wide grid (cols 110/111 per row are
# junk, sliced off on host). 24 blocks of 512 positions + 1 tail block of 64
# covers all valid positions (max valid flat pos = 109*112+109 = 12317).
NPOS = HO * W                    # 12320
BLK = 512
NBLK_FULL = 24                   # 24*512 = 12288
TAIL = 64                        # positions 12288..12352
BLOCKS = [(b * BLK, BLK) for b in range(NBLK_FULL)] + [(NBLK_FULL * BLK, TAIL)]
OUT_W = NBLK_FULL * BLK + TAIL   # 12352 stored positions per image
XT_W = 12608                     # >= 12288 + 226 + 64; tail zeroed

_F32 = mybir.dt.float32
_BF16 = mybir.dt.bfloat16


def _split_waits(nc, maxw=1):
    """walrus in this container rejects multiple sync-waits per instruction
    (observed on Drain and fused-LDW Matmult). Move overflow waits onto
    NoOps inserted just before the instruction — semantically identical,
    the sequencer blocks between the nop and the instruction either way."""
    def limit(inst):
        return maxw

    for f in nc.m.functions:
        for bb in f.blocks:
            new_insts = []
            for inst in bb.instructions:
                si = inst.sync_info
                mw = limit(inst)
                if si is not None and si.on_wait and len(si.on_wait) > mw:
                    waits = list(si.on_wait)
                    overflow, keep = waits[:-mw], waits[-mw:]
                    for ci in range(0, len(overflow), 1):
                        nop = mybir.InstNoOp(
                            name=f"{inst.name}-ws{ci}",
                            engine=inst.engine,
                            ins=[], outs=[],
                            sync_info=mybir.SyncInfo(
                                on_wait=overflow[ci:ci + 1], on_update=[]),
                        )
                        nc.register_instruction(nop, overwrite=True)
                        new_insts.append(nop)
                    inst.sync_info = mybir.SyncInfo(
                        on_wait=keep, on_update=list(si.on_update or []))
                new_insts.append(inst)
            bb.instructions[:] = new_insts


def build_nc():
    nc = bass.Bass("TRN2", target_bir_lowering=False, debug=False,
                   num_devices=N_CORES, num_swdge_queues=2)

    xt_d = nc.dram_tensor("xt_shard", [NPC, CIN, PIX], _BF16,
                          kind="ExternalInput")
    wb_d = nc.dram_tensor("wb_packed", [CIN, NTAP * COUT], _BF16,
                          kind="ExternalInput")
    b_d = nc.dram_tensor("bias_cols", [128, 2], _F32, kind="ExternalInput")
    # output stored transposed: [n, co_half, co_lane, pix]; host restores NHWC
    o_d = nc.dram_tensor("out", [NPC, 2, 128, OUT_W], _BF16,
                         kind="ExternalOutput")

    with tile.TileContext(nc) as tc, ExitStack() as ctx:
        const_pool = ctx.enter_context(tc.tile_pool(name="const", bufs=1))
        xt_pool = ctx.enter_context(tc.tile_pool(name="xt", bufs=2))
        out_pool = ctx.enter_context(tc.tile_pool(name="osb", bufs=6))
        psc_pool = ctx.enter_context(
            tc.tile_pool(name="psc", bufs=4, space="PSUM"))

        bias_sb = const_pool.tile([128, 2], _F32, tag="bias")
        nc.sync.dma_start(bias_sb[:], b_d.ap()[:])
        wb = const_pool.tile([128, NTAP * COUT], _BF16, tag="wb")
        nc.sync.dma_start(wb[:], wb_d.ap()[:])

        for n in range(NPC):
            # pre-transposed image: one clean 25KB/partition DMA (ACT ring,
            # so loads never queue behind the output stores on the SP ring)
            xt = xt_pool.tile([128, XT_W], _BF16, tag="xt")
            nc.vector.memset(xt[:, PIX:XT_W], 0.0)
            nc.scalar.dma_start(xt[:, :PIX], xt_d.ap()[n])

            # conv: per (block, half) psum tile, 9 tap matmuls, then the
            # bias add fused into the PSUM->SBUF copy (alternating DVE/ACT)
            for bi, (s, blen) in enumerate(BLOCKS):
                for h in range(2):
                    psc = psc_pool.tile([128, blen], _F32, tag="psc",
                                        name="psc")
                    for tap in range(NTAP):
                        toff = (tap // KW) * W + (tap % KW)
                        wsl = wb[:, tap * COUT + h * 128:
                                 tap * COUT + h * 128 + 128]
                        nc.tensor.matmul(
                            psc[:, :], wsl, xt[:, s + toff:s + toff + blen],
                            start=(tap == 0), stop=(tap == NTAP - 1))
                    osb = out_pool.tile([128, blen], _BF16, tag="osb")
                    if (bi + h) % 2 == 0:
                        nc.vector.tensor_scalar(
                            osb[:], psc[:], bias_sb[:, h:h + 1],
                            None, mybir.AluOpType.add)
                    else:
                        nc.scalar.add(osb[:], psc[:], bias_sb[:, h:h + 1])
                    nc.sync.dma_start(
                        o_d.ap()[n, h, :, s:s + blen], osb[:, :])

    _split_waits(nc)
    return nc


_NC_CACHE = None
LAST_RESULTS = None  # BassKernelResults of the most recent kernel() call


def _get_nc():
    global _NC_CACHE
    if _NC_CACHE is None:
        _NC_CACHE = build_nc()
    return _NC_CACHE


def kernel(x: np.ndarray, kernel: np.ndarray, bias: np.ndarray) -> np.ndarray:
    global LAST_RESULTS
    nc = _get_nc()

    # host-side layout prep (sharding/packing, no math beyond cast/compare):
    # x -> per-image [ci, pix] bf16; kernel -> binarized [ci, (tap co)] bf16
    xt = np.ascontiguousarray(
        x.astype(ml_dtypes.bfloat16).reshape(N, PIX, CIN).transpose(0, 2, 1))
    wb = np.where(kernel.astype(np.float32) + np.float32(1.0)
                  > np.float32(1.0), np.float32(1.0), np.float32(-1.0))
    wb = np.ascontiguousarray(
        wb.transpose(2, 0, 1, 3).reshape(CIN, NTAP * COUT)
    ).astype(ml_dtypes.bfloat16)
    bias_cols = np.ascontiguousarray(
        bias.astype(np.float32).reshape(2, 128).T)

    in_maps = [
        {
            "xt_shard": np.ascontiguousarray(xt[c * NPC:(c + 1) * NPC]),
            "wb_packed": wb,
            "bias_cols": bias_cols,
        }
        for c in range(N_CORES)
    ]
    trace = bool(os.environ.get("KERNEL_TRACE"))
    res = run_bass_kernel_spmd(nc, in_maps, list(range(N_CORES)),
                               trace=trace)
    LAST_RESULTS = res
    parts = []
    for c in range(N_CORES):
        o = res.results[c]["out"]  # [NPC, 2, 128, OUT_W] bf16, [co, pix]
        o = np.asarray(o).astype(np.float32).reshape(NPC, COUT, OUT_W)
        o = o[:, :, :NPOS].reshape(NPC, COUT, HO, W)[:, :, :, :WO]
        parts.append(np.ascontiguousarray(o.transpose(0, 2, 3, 1)))
    return np.ascontiguousarray(np.concatenate(parts, axis=0),
                                dtype=np.float32)
